# revision 1
# baseline (speedup 1.0000x reference)
"""Trainium2 Bass kernel for windowed multi-head attention (ClassicAttention).

Shapes (hardcoded per spec): x (1024, 68, 768), pe (128, 768), mask zeros.
Data-parallel over 8 NeuronCores on the leading window axis.

v2: bf16 matmul operands throughout (fp32 moving operands cost 2-4
cycles/row on the PE; bf16 costs 1), pe folded into x on the host,
biases folded into PSUM->SBUF copies / proj bias, softmax normalization
applied post-AV via a broadcast multiply (drops the ESN multiply and
the slow single-lane reciprocal in the critical chain;
reciprocal_approx_fast is ~5x faster and accurate to ~18 bits).
"""

import os
import sys

for _p in (
    "/root/.axon_site",
    "/root/.axon_site/_ro/trn_rl_repo",
    "/root/.axon_site/_ro/pypackages",
    "/opt/trn_rl_repo",
):
    if os.path.isdir(_p) and _p not in sys.path:
        sys.path.append(_p)

import ml_dtypes
import numpy as np

import concourse.bass as bass
import concourse.mybir as mybir
import concourse.tile as tile
from concourse import bacc
from concourse.bass_utils import run_bass_kernel_spmd

F32 = mybir.dt.float32
BF16 = mybir.dt.bfloat16
AFT = mybir.ActivationFunctionType

NCORES = 8
B_, N, C = 1024, 68, 768
H, HD = 12, 64
N_VTS = 4
KT = C // 128            # 6 contraction tiles of 128
BL = B_ // NCORES        # 128 windows per core
G = 4                    # windows per group
NG = BL // G             # 32 groups
FD = G * N               # 272

_CACHE = {}


def _build_nc():
    nc = bacc.Bacc(trn_type="TRN2", target_bir_lowering=False, debug=False)

    xt_d = nc.dram_tensor("xt", [128, KT, BL, N], BF16, kind="ExternalInput")
    w1_d = nc.dram_tensor("w1", [128, 12, KT, 128], BF16, kind="ExternalInput")
    w2_d = nc.dram_tensor("w2", [128, KT, C], BF16, kind="ExternalInput")
    wp_d = nc.dram_tensor("wp", [128, KT, KT, 128], BF16, kind="ExternalInput")
    bqk_d = nc.dram_tensor("bqk", [128, 12], F32, kind="ExternalInput")
    bqkb_d = nc.dram_tensor("bqkb", [128, 12, FD], BF16, kind="ExternalInput")
    bpb_d = nc.dram_tensor("bpb", [128, KT, FD], F32, kind="ExternalInput")
    ones68_d = nc.dram_tensor("ones68", [N, 1], BF16, kind="ExternalInput")
    out_d = nc.dram_tensor("outt", [128, KT, BL, N], F32, kind="ExternalOutput")

    with tile.TileContext(nc) as tc:
        with (
            tc.tile_pool(name="wgt", bufs=1) as wp_pool,
            tc.tile_pool(name="xp", bufs=2) as xp,
            tc.tile_pool(name="qkp", bufs=2) as qkp,
            tc.tile_pool(name="vp", bufs=2) as vp,
            tc.tile_pool(name="esp", bufs=6) as esp,
            tc.tile_pool(name="atp", bufs=2) as atp,
            tc.tile_pool(name="rp", bufs=4) as rp,
            tc.tile_pool(name="rbp", bufs=6) as rbp,
            tc.tile_pool(name="otp", bufs=3) as otp,
            tc.tile_pool(name="pbig", bufs=2, space="PSUM") as pbig,
            tc.tile_pool(name="ppv", bufs=1, space="PSUM") as ppv,
            tc.tile_pool(name="psc", bufs=2, space="PSUM") as psc,
            tc.tile_pool(name="ps1", bufs=1, space="PSUM") as ps1p,
            tc.tile_pool(name="pav", bufs=1, space="PSUM") as pavp,
        ):
            W1s = wp_pool.tile([128, 12, KT, 128], BF16)
            W2s = wp_pool.tile([128, KT, C], BF16)
            WPs = wp_pool.tile([128, KT, KT, 128], BF16)
            BQKs = wp_pool.tile([128, 12], F32)
            BQKB = wp_pool.tile([128, 12, FD], BF16)
            # proj bias pre-broadcast along the free dim (host-built), so the
            # OT drain can run on the vector engine (scalar's in-order queue
            # otherwise convoys the next group's QKT copies behind proj waits)
            BPB = wp_pool.tile([128, KT, FD], F32)
            ONES68s = wp_pool.tile([N, 1], BF16)
            nc.sync.dma_start(W1s[:], w1_d.ap())
            nc.sync.dma_start(W2s[:], w2_d.ap())
            nc.sync.dma_start(WPs[:], wp_d.ap())
            nc.sync.dma_start(BQKs[:], bqk_d.ap())
            nc.sync.dma_start(BQKB[:], bqkb_d.ap())
            nc.sync.dma_start(BPB[:], bpb_d.ap())
            nc.sync.dma_start(ONES68s[:], ones68_d.ap())

            for g in range(NG):
                gsl = slice(G * g, G * (g + 1))
                XT = xp.tile([128, KT, G, N], BF16, tag="xt")
                nc.gpsimd.dma_start(XT[:], xt_d.ap()[:, :, gsl, :])

                # ---- q,k in transposed layout: QKT[p, j, w, t] (j<6: q, j>=6: k)
                QKT = qkp.tile([128, 12, G, N], BF16, tag="qkt")
                for j in range(12):
                    pq = pbig.tile([128, FD], F32, tag="big")
                    for k in range(KT):
                        nc.tensor.matmul(
                            pq[:], W1s[:, j, k, :], XT[:, k, :, :],
                            start=(k == 0), stop=(k == KT - 1),
                        )
                    qsrc = pq.rearrange("p (a b) -> p a b", a=G)
                    qbias = BQKB.rearrange(
                        "p j (a b) -> p j a b", a=G)[:, j, :, :]
                    if j % 2 == 0:
                        nc.scalar.activation(
                            QKT[:, j, :, :], qsrc, AFT.Identity,
                            bias=BQKs[:, j:j + 1],
                        )
                    else:
                        nc.vector.tensor_add(QKT[:, j, :, :], qsrc, qbias)

                # ---- attention scores + exp + sums, per window
                ES = {}
                S1 = {}

                def qk_exp(w):
                    # ES slot s = 6*half + hh holds head h = 2*hh + half, so
                    # each PSUM bank sees a single PE row-group (HW hangs on
                    # mixed-row-group matmuls into one bank).
                    ES[w] = esp.tile([N, H, N], BF16, tag="es", name="es")
                    for half in range(2):
                        sc = psc.tile([N, 6, N], F32, tag="sc")
                        p0 = 64 * half
                        for hh in range(6):
                            nc.tensor.matmul(
                                sc[:, hh, :],
                                QKT[p0:p0 + 64, 6 + hh, w, :],
                                QKT[p0:p0 + 64, hh, w, :],
                                start=True, stop=True, skip_group_check=True,
                            )
                        nc.scalar.activation(
                            ES[w][:, 6 * half:6 * half + 6, :], sc[:], AFT.Exp
                        )

                R = {}

                def sums(w):
                    # both halves' sums at partition 0 (custom-DVE ops require
                    # partition-0 APs); the 512-pad keeps each half's 408-col
                    # matmul output inside a single PSUM bank
                    S1[w] = ps1p.tile([1, 2, 512], F32, tag="s1", name="s1")
                    for half in range(2):
                        nc.tensor.matmul(
                            S1[w][0:1, half, 0:6 * N],
                            ONES68s[:],
                            ES[w][:, 6 * half:6 * half + 6, :],
                            start=True, stop=True, skip_group_check=True,
                        )
                    # 1/rowsum immediately (vector), so the ps1 PSUM banks
                    # recycle without stalling the tensor queue, then
                    # broadcast to all 128 partitions for the post-AV scale
                    r1 = rp.tile([1, 2, 6 * N], F32, tag="r1")
                    for half in range(2):
                        nc.vector.reciprocal_approx_fast(
                            r1[0:1, half, :],
                            S1[w][0:1, half, 0:6 * N],
                        )
                    R[w] = rbp.tile([128, 6, N], F32, tag="rb", name="rb")
                    for half, eng in ((0, nc.sync), (1, nc.sync)):
                        row = r1[0:1, half, :]
                        src = bass.AP(
                            tensor=row.tensor, offset=row.offset,
                            ap=[list(row.ap[0]), [0, 64], [1, 6 * N]],
                        )
                        eng.dma_start(R[w][64 * half:64 * half + 64, :, :], src)

                for w in range(G):
                    qk_exp(w)
                    if w >= 1:
                        sums(w - 1)
                sums(G - 1)

                # ---- v in direct layout: V[t, w, o] (one PSUM bank, two
                # sequential half-GEMMs per window)
                V = vp.tile([N, G, C], BF16, tag="v")
                for w in range(G):
                    for hs in (slice(0, 512), slice(512, C)):
                        pv = ppv.tile([N, 512], F32, tag="pv")
                        hw_ = hs.stop - hs.start
                        for k in range(KT):
                            nc.tensor.matmul(
                                pv[:, 0:hw_], XT[:, k, w, :], W2s[:, k, hs],
                                start=(k == 0), stop=(k == KT - 1),
                            )
                        nc.vector.tensor_copy(V[:, w, hs], pv[:, 0:hw_])

                # ---- AV into AT[o, kt, w, t], normalized during PSUM drain
                AT = atp.tile([128, KT, G, N], BF16, tag="at")
                for w in range(G):
                    pa = pavp.tile([128, KT, N], F32, tag="pa")
                    for s in range(H):
                        h = 2 * (s % 6) + (s // 6)  # head held in ES slot s
                        nc.tensor.matmul(
                            pa[64 * (s // 6):64 * (s // 6) + 64, s % 6, :],
                            V[:, w, 64 * h:64 * (h + 1)],
                            ES[w][:, s, :],
                            start=True, stop=True, skip_group_check=True,
                        )
                    nc.vector.tensor_mul(AT[:, :, w, :], pa[:], R[w][:])

                # ---- proj: out^T[o, w, t] with bias folded into the drain
                for j in range(KT):
                    po = pbig.tile([128, FD], F32, tag="big")
                    for kt in range(KT):
                        nc.tensor.matmul(
                            po[:], WPs[:, j, kt, :], AT[:, kt, :, :],
                            start=(kt == 0), stop=(kt == KT - 1),
                        )
                    OT = otp.tile([128, G, N], F32, tag="ot")
                    nc.vector.tensor_add(
                        OT[:], po.rearrange("p (a b) -> p a b", a=G),
                        BPB.rearrange("p j (a b) -> p j a b", a=G)[:, j, :, :],
                    )
                    nc.gpsimd.dma_start(out_d.ap()[:, j, gsl, :], OT[:])

    nc.compile()
    return nc


def _host_prep(x, pe, w_qkv, b_qkv, w_proj, b_proj):
    f = np.float32
    bf = ml_dtypes.bfloat16
    x = np.asarray(x, f)
    pe = np.asarray(pe, f)
    w_qkv = np.asarray(w_qkv, f)
    b_qkv = np.asarray(b_qkv, f)
    w_proj = np.asarray(w_proj, f)
    b_proj = np.asarray(b_proj, f)

    scale = f(HD ** -0.5)
    n_ = N - N_VTS
    strt = pe.shape[0] // 2 - n_ // 2

    # fold pe into x on the host; shard and transpose to [p, k, b, t]
    xp = x.copy()
    xp[:, N_VTS:, :] += pe[strt:strt + n_]
    xt = np.ascontiguousarray(
        xp.reshape(NCORES, BL, N, KT, 128).transpose(0, 4, 3, 1, 2)).astype(bf)

    w_qk = np.concatenate([w_qkv[:C] * scale, w_qkv[C:2 * C]], axis=0)  # (1536, 768)
    W1 = np.ascontiguousarray(
        w_qk.reshape(12, 128, KT, 128).transpose(3, 0, 2, 1)).astype(bf)  # [p,j,k,o]
    W2 = np.ascontiguousarray(
        w_qkv[2 * C:].reshape(C, KT, 128).transpose(2, 1, 0)).astype(bf)  # [p,k,o]
    WP = np.ascontiguousarray(
        w_proj.reshape(KT, 128, KT, 128).transpose(3, 0, 2, 1)).astype(bf)  # [p,j,kt,o]

    b_qk = np.concatenate([b_qkv[:C] * scale, b_qkv[C:2 * C]])
    BQK = np.ascontiguousarray(b_qk.reshape(12, 128).T).astype(f)        # [p, j]
    BQKB = np.ascontiguousarray(np.broadcast_to(
        BQK[:, :, None], (128, 12, FD))).astype(bf)                      # [p, j, fd]
    # b_v folds into the proj bias: softmax rows sum to 1, so
    # proj(attn @ (v + b_v)) = proj(attn @ v) + w_proj @ b_v
    bpp = b_proj + w_proj @ b_qkv[2 * C:]
    BPB = np.ascontiguousarray(np.broadcast_to(
        bpp.reshape(KT, 128).T[:, :, None], (128, KT, FD))).astype(f)    # [p, j, fd]
    ONES68 = np.ones((N, 1), bf)

    shared = {
        "w1": W1, "w2": W2, "wp": WP, "bqk": BQK, "bqkb": BQKB, "bpb": BPB,
        "ones68": ONES68,
    }
    return xt, shared


def kernel(x, pe, mask, w_qkv, b_qkv, w_proj, b_proj):
    del mask  # zeros by problem spec
    xt, shared = _host_prep(x, pe, w_qkv, b_qkv, w_proj, b_proj)

    if "nc" not in _CACHE:
        _CACHE["nc"] = _build_nc()
    nc = _CACHE["nc"]

    in_maps = [dict(shared, xt=xt[c]) for c in range(NCORES)]
    res = run_bass_kernel_spmd(
        nc, in_maps, core_ids=list(range(NCORES)),
        **_CACHE.get("run_kwargs", {}),
    )
    _CACHE["last_result"] = res

    # outt [core, p, j, b, t] -> (1024, 68, 768)
    outt = np.stack([res.results[c]["outt"] for c in range(NCORES)])
    out = np.ascontiguousarray(
        outt.transpose(0, 3, 4, 2, 1).reshape(B_, N, C))
    return out



# revision 4
# speedup vs baseline: 1.6749x; 1.6749x over previous
"""Trainium2 Bass kernel for windowed multi-head attention (ClassicAttention).

Shapes (hardcoded per spec): x (1024, 68, 768), pe (128, 768), mask zeros.
Data-parallel over 8 NeuronCores on the leading window axis.

v3 (from v2 baseline @1719us):
- The per-window 1/rowsum broadcast DMA (partition-stride-0 source -> ~64
  descriptors each, flooding all 16 DMA queues and stalling the PE ~6us per
  group) is replaced by a rowsum matmul whose stationary is an all-ones
  [68, 64] tile: the PSUM output [128, 408] holds the row sums already
  replicated across partitions, and the reciprocal drains it straight to
  the SBUF R tile. No DMA in the softmax-normalize chain.
- V is computed weights-stationary / x-moving ([128,128] x [128,272] per
  (o-tile, k), 272-cycle matmuls) instead of x-stationary / W-moving
  (512-cycle matmuls re-streaming W per window), then PE-transposed per
  (window, o-tile) into the [token, channel] layout AV needs. Saves ~5us
  of PE time per group.
- Contiguous DRAM layouts for the per-group x slab and output slab
  (1 descriptor chunk per partition instead of 6/24 strided ones).
- x loads on the sync queue (decoupled from output stores on the Pool
  queue); weight loads split per output tile so they spread across DMA
  queues and the first group's QK-gen can start on subtile deps.
- Drains spread across Act/DVE/Pool so no single engine queue paces the PE.
"""

import os
import sys

for _p in (
    "/root/.axon_site",
    "/root/.axon_site/_ro/trn_rl_repo",
    "/root/.axon_site/_ro/pypackages",
    "/opt/trn_rl_repo",
):
    if os.path.isdir(_p) and _p not in sys.path:
        sys.path.append(_p)

import ml_dtypes
import numpy as np

import concourse.bass as bass
import concourse.mybir as mybir
import concourse.tile as tile
from concourse import bacc
from concourse.bass_utils import run_bass_kernel_spmd

F32 = mybir.dt.float32
BF16 = mybir.dt.bfloat16
AFT = mybir.ActivationFunctionType

NCORES = 8
B_, N, C = 1024, 68, 768
H, HD = 12, 64
N_VTS = 4
KT = C // 128            # 6 contraction tiles of 128
BL = B_ // NCORES        # 128 windows per core
G = 4                    # windows per group
NG = BL // G             # 32 groups
FD = G * N               # 272

_CACHE = {}


def _build_nc():
    nc = bacc.Bacc(trn_type="TRN2", target_bir_lowering=False, debug=False)

    xt_d = nc.dram_tensor("xt", [128, NG, KT, G, N], BF16, kind="ExternalInput")
    w1_d = nc.dram_tensor("w1", [128, 12, KT, 128], BF16, kind="ExternalInput")
    w2_d = nc.dram_tensor("w2", [128, KT, KT, 128], BF16, kind="ExternalInput")
    wp_d = nc.dram_tensor("wp", [128, KT, KT, 128], BF16, kind="ExternalInput")
    bqk_d = nc.dram_tensor("bqk", [128, 12], F32, kind="ExternalInput")
    bqkb_d = nc.dram_tensor("bqkb", [128, 12, FD], BF16, kind="ExternalInput")
    bpb_d = nc.dram_tensor("bpb", [128, KT, FD], F32, kind="ExternalInput")
    ones_d = nc.dram_tensor("ones", [N, 64], BF16, kind="ExternalInput")
    idt_d = nc.dram_tensor("idt", [128, 128], BF16, kind="ExternalInput")
    out_d = nc.dram_tensor("outt", [128, NG, KT, G, N], F32, kind="ExternalOutput")

    with tile.TileContext(nc) as tc:
        with (
            tc.tile_pool(name="wgt", bufs=1) as wp_pool,
            tc.tile_pool(name="xp", bufs=3) as xp,
            tc.tile_pool(name="qkp", bufs=2) as qkp,
            tc.tile_pool(name="vtp", bufs=2) as vtp,
            # all 4 windows' V tiles are live at once (AV(w0) must not wait on
            # copy(V2) via slot reuse - that cycles with the shared trpa PSUM
            # slots); 5th buf decouples the next group's first copy
            tc.tile_pool(name="vp", bufs=5) as vp,
            tc.tile_pool(name="esp", bufs=6) as esp,
            tc.tile_pool(name="atp", bufs=2) as atp,
            tc.tile_pool(name="rbp", bufs=6) as rbp,
            tc.tile_pool(name="otp", bufs=3) as otp,
            tc.tile_pool(name="pbig", bufs=2, space="PSUM") as pbig,
            tc.tile_pool(name="psc", bufs=2, space="PSUM") as psc,
            tc.tile_pool(name="prs", bufs=2, space="PSUM") as prs,
            tc.tile_pool(name="ptp", bufs=2, space="PSUM") as ptp,
        ):
            W1s = wp_pool.tile([128, 12, KT, 128], BF16)
            W2s = wp_pool.tile([128, KT, KT, 128], BF16)
            WPs = wp_pool.tile([128, KT, KT, 128], BF16)
            BQKs = wp_pool.tile([128, 12], F32)
            BQKB = wp_pool.tile([128, 12, FD], BF16)
            # proj bias pre-broadcast along the free dim (host-built)
            BPB = wp_pool.tile([128, KT, FD], F32)
            ONES = wp_pool.tile([N, 64], BF16)
            IDT = wp_pool.tile([128, 128], BF16)
            # split the big weight loads so they spread across DMA queues and
            # subtile deps let the first QK-gen start before all 12 j arrive
            for j in range(12):
                nc.sync.dma_start(W1s[:, j, :, :], w1_d.ap()[:, j, :, :])
            for ot in range(KT):
                nc.sync.dma_start(W2s[:, ot, :, :], w2_d.ap()[:, ot, :, :])
            for j in range(KT):
                nc.sync.dma_start(WPs[:, j, :, :], wp_d.ap()[:, j, :, :])
            nc.sync.dma_start(BQKs[:], bqk_d.ap())
            nc.sync.dma_start(BQKB[:], bqkb_d.ap())
            nc.sync.dma_start(BPB[:], bpb_d.ap())
            nc.sync.dma_start(ONES[:], ones_d.ap())
            nc.sync.dma_start(IDT[:], idt_d.ap())

            for g in range(NG):
                XT = xp.tile([128, KT, G, N], BF16, tag="xt")
                nc.sync.dma_start(XT[:], xt_d.ap()[:, g, :, :, :])

                # ---- q,k in transposed layout: QKT[p, j, w, t] (j<6: q, j>=6: k)
                QKT = qkp.tile([128, 12, G, N], BF16, tag="qkt")
                for j in range(12):
                    pq = pbig.tile([128, FD], F32, tag="big")
                    for k in range(KT):
                        nc.tensor.matmul(
                            pq[:], W1s[:, j, k, :], XT[:, k, :, :],
                            start=(k == 0), stop=(k == KT - 1),
                        )
                    qsrc = pq.rearrange("p (a b) -> p a b", a=G)
                    if j % 2 == 0:
                        nc.scalar.activation(
                            QKT[:, j, :, :], qsrc, AFT.Identity,
                            bias=BQKs[:, j:j + 1],
                        )
                    else:
                        qbias = BQKB.rearrange(
                            "p j (a b) -> p j a b", a=G)[:, j, :, :]
                        nc.vector.tensor_add(QKT[:, j, :, :], qsrc, qbias)

                # ---- attention scores + exp, per window
                # ES slot s = 6*half + hh holds head h = 2*hh + half, so
                # each PSUM bank sees a single PE row-group (HW hangs on
                # mixed-row-group matmuls into one bank).
                ES = {}
                for w in range(G):
                    ES[w] = esp.tile([N, H, N], BF16, tag="es", name="es")
                    for half in range(2):
                        sc = psc.tile([N, 6, N], F32, tag="sc")
                        p0 = 64 * half
                        for hh in range(6):
                            nc.tensor.matmul(
                                sc[:, hh, :],
                                QKT[p0:p0 + 64, 6 + hh, w, :],
                                QKT[p0:p0 + 64, hh, w, :],
                                start=True, stop=True, skip_group_check=True,
                            )
                        nc.scalar.activation(
                            ES[w][:, 6 * half:6 * half + 6, :], sc[:], AFT.Exp
                        )

                # ---- v^T[o, ot, w, t]: weights stationary, x moving
                VT = vtp.tile([128, KT, G, N], BF16, tag="vt")
                for ot in range(KT):
                    pv = pbig.tile([128, FD], F32, tag="big")
                    for k in range(KT):
                        nc.tensor.matmul(
                            pv[:], W2s[:, ot, k, :], XT[:, k, :, :],
                            start=(k == 0), stop=(k == KT - 1),
                        )
                    nc.vector.tensor_copy(
                        VT[:, ot, :, :], pv.rearrange("p (a b) -> p a b", a=G))

                # ---- per window: rowsums (replicated via all-ones stationary),
                # reciprocal, V transpose to [token, channel]
                R = {}
                V = {}
                for w in range(G):
                    rs = prs.tile([128, 512], F32, tag="rs", name="rs")
                    for half in range(2):
                        nc.tensor.matmul(
                            rs[64 * half:64 * half + 64, 0:6 * N],
                            ONES[:],
                            ES[w][:, 6 * half:6 * half + 6, :],
                            start=True, stop=True, skip_group_check=True,
                        )
                    R[w] = rbp.tile([128, 6, N], F32, tag="rb", name="rb")
                    nc.vector.reciprocal_approx_fast(R[w][:], rs[:, 0:6 * N])

                    tr = ptp.tile([N, KT, 128], BF16, tag="trpa", name="tr")
                    for ot in range(KT):
                        nc.tensor.matmul(
                            tr[:, ot, :], VT[:, ot, w, :], IDT[:],
                            is_transpose=True, skip_group_check=True,
                        )
                    V[w] = vp.tile([N, KT, 128], BF16, tag="v", name="v")
                    nc.scalar.copy(V[w][:], tr[:])

                # ---- AV into AT[o, kt, w, t], normalized during PSUM drain
                AT = atp.tile([128, KT, G, N], BF16, tag="at")
                for w in range(G):
                    pa = ptp.tile([128, KT, N], F32, tag="trpa", name="pa")
                    for s in range(H):
                        h = 2 * (s % 6) + (s // 6)  # head held in ES slot s
                        nc.tensor.matmul(
                            pa[64 * (s // 6):64 * (s // 6) + 64, s % 6, :],
                            V[w][:, h // 2, 64 * (h % 2):64 * (h % 2) + 64],
                            ES[w][:, s, :],
                            start=True, stop=True, skip_group_check=True,
                        )
                    nc.vector.tensor_mul(AT[:, :, w, :], pa[:], R[w][:])

                # ---- proj: out^T[o, w, t] with bias folded into the drain
                for j in range(KT):
                    po = pbig.tile([128, FD], F32, tag="big")
                    for kt in range(KT):
                        nc.tensor.matmul(
                            po[:], WPs[:, j, kt, :], AT[:, kt, :, :],
                            start=(kt == 0), stop=(kt == KT - 1),
                        )
                    OT = otp.tile([128, G, N], F32, tag="ot")
                    # NOT on gpsimd: the Pool engine cannot access PSUM
                    nc.vector.tensor_add(
                        OT[:], po.rearrange("p (a b) -> p a b", a=G),
                        BPB.rearrange("p j (a b) -> p j a b", a=G)[:, j, :, :],
                    )
                    nc.gpsimd.dma_start(out_d.ap()[:, g, j, :, :], OT[:])

    nc.compile()
    return nc


def _host_prep(x, pe, w_qkv, b_qkv, w_proj, b_proj):
    f = np.float32
    bf = ml_dtypes.bfloat16
    x = np.asarray(x, f)
    pe = np.asarray(pe, f)
    w_qkv = np.asarray(w_qkv, f)
    b_qkv = np.asarray(b_qkv, f)
    w_proj = np.asarray(w_proj, f)
    b_proj = np.asarray(b_proj, f)

    scale = f(HD ** -0.5)
    n_ = N - N_VTS
    strt = pe.shape[0] // 2 - n_ // 2

    # fold pe into x on the host; shard and transpose to [p, g, k, w, t]
    xp = x.copy()
    xp[:, N_VTS:, :] += pe[strt:strt + n_]
    xt = np.ascontiguousarray(
        xp.reshape(NCORES, NG, G, N, KT, 128).transpose(0, 5, 1, 4, 2, 3)
    ).astype(bf)

    w_qk = np.concatenate([w_qkv[:C] * scale, w_qkv[C:2 * C]], axis=0)  # (1536, 768)
    W1 = np.ascontiguousarray(
        w_qk.reshape(12, 128, KT, 128).transpose(3, 0, 2, 1)).astype(bf)  # [p,j,k,o]
    W2 = np.ascontiguousarray(
        w_qkv[2 * C:].reshape(KT, 128, KT, 128).transpose(3, 0, 2, 1)
    ).astype(bf)                                                          # [p,ot,k,o]
    WP = np.ascontiguousarray(
        w_proj.reshape(KT, 128, KT, 128).transpose(3, 0, 2, 1)).astype(bf)  # [p,j,kt,o]

    b_qk = np.concatenate([b_qkv[:C] * scale, b_qkv[C:2 * C]])
    BQK = np.ascontiguousarray(b_qk.reshape(12, 128).T).astype(f)        # [p, j]
    BQKB = np.ascontiguousarray(np.broadcast_to(
        BQK[:, :, None], (128, 12, FD))).astype(bf)                      # [p, j, fd]
    # b_v folds into the proj bias: softmax rows sum to 1, so
    # proj(attn @ (v + b_v)) = proj(attn @ v) + w_proj @ b_v
    bpp = b_proj + w_proj @ b_qkv[2 * C:]
    BPB = np.ascontiguousarray(np.broadcast_to(
        bpp.reshape(KT, 128).T[:, :, None], (128, KT, FD))).astype(f)    # [p, j, fd]
    ONES = np.ones((N, 64), bf)
    IDT = np.eye(128, dtype=bf)

    shared = {
        "w1": W1, "w2": W2, "wp": WP, "bqk": BQK, "bqkb": BQKB, "bpb": BPB,
        "ones": ONES, "idt": IDT,
    }
    return xt, shared


def kernel(x, pe, mask, w_qkv, b_qkv, w_proj, b_proj):
    del mask  # zeros by problem spec
    xt, shared = _host_prep(x, pe, w_qkv, b_qkv, w_proj, b_proj)

    if "nc" not in _CACHE:
        _CACHE["nc"] = _build_nc()
    nc = _CACHE["nc"]

    in_maps = [dict(shared, xt=xt[c]) for c in range(NCORES)]
    res = run_bass_kernel_spmd(
        nc, in_maps, core_ids=list(range(NCORES)),
        **_CACHE.get("run_kwargs", {}),
    )
    _CACHE["last_result"] = res

    # outt [core, p, g, j, w, t] -> (1024, 68, 768)
    outt = np.stack([res.results[c]["outt"] for c in range(NCORES)])
    out = np.ascontiguousarray(
        outt.transpose(0, 2, 4, 5, 3, 1).reshape(B_, N, C))
    return out
